# revision 9
# baseline (speedup 1.0000x reference)
"""Causal self-attention (GPT-style block) on 8 Trainium2 NeuronCores.

Problem: x[4, 2048, 768], w_attn[2304, 768], b_attn[2304], w_proj[768, 768],
b_proj[768]; 12 heads of size 64; causal softmax attention; output [4, 2048, 768].

Sharding: batch x heads. core = 2*b + g handles batch b (of 4) and the 6 heads
g*6..g*6+5 (tensor parallel over heads). Host-side staging is minimized: every
in_maps entry is a view of the original arrays (no host transposes), each byte
of x and of the weights is uploaded to exactly one core, and the output comes
back already reduced and bias-added, so host assembly is a reshape.

Per core:
  0. Prep: AllGather x halves within batch pairs (cores 2b,2b+1) to recover
     x[b]; AllGather weight quarter-slices within head-group quads
     ({0,2,4,6}/{1,3,5,7}) to recover this group's w_qk/w_v/w_proj slices.
     Transpose x and the weights on the PE (identity matmul) into the [c, t]
     / [c, r] layouts the matmuls need; tri/ones/identity constants are
     generated on device (affine_select / memset).
  1. QKV projection for its head slice, producing Q^T/K^T in [r, t] layout and
     V in [t, r] layout (plus a fused ones column for softmax denominators).
  2. Flash-style causal attention per head: S^T tiles [128 kv, 512 q] via PE,
     exp on ACT (scale=1/8), triangular mask on the diagonal 128x128 block via
     DVE, O^T accumulation on PE with the V-ones column yielding the softmax
     denominator for free, then per-column normalization via a rank-1
     broadcast matmul + DVE multiply.
  3. c_proj with its 384 local channels (+ b_proj, fed only to even cores so
     the pair-sum applies it once) -> partial y[2048, 768] in DRAM.
  4. ReduceScatter over the pair sums the partials; core 2b+g ends with tokens
     g*1024..(g+1)*1024 of y[b]. The global concat over cores is exactly
     y.reshape(4*2048, 768).

All matmuls run as float32r (TF32-like, 1 cycle/row at N>=256) with fp32 PSUM
accumulation.
"""
import os

import numpy as np

os.environ.setdefault("JAX_COMPILATION_CACHE_DIR", "/tmp/jaxcache")
os.environ.setdefault("JAX_PERSISTENT_CACHE_MIN_COMPILE_TIME_SECS", "0")
os.environ.setdefault("JAX_PERSISTENT_CACHE_MIN_ENTRY_SIZE_BYTES", "0")

import concourse.bass as bass
import concourse.bacc as bacc
import concourse.tile as tile
from concourse import mybir
from concourse.bass_utils import run_bass_kernel_spmd

B, T, C, H = 4, 2048, 768, 12
HS = 64          # head size
HL = 6           # heads per core
CL = HL * HS     # 384 local channels per core
TH = T // 2      # tokens staged per core
NQ = 512         # q block width
NCH = T // NQ    # 4 chunks
NKB = T // 128   # 16 kv blocks
NCORES = 8
F32 = mybir.dt.float32
F32R = mybir.dt.float32r
EXP = mybir.ActivationFunctionType.Exp
PAIRS = [[0, 1], [2, 3], [4, 5], [6, 7]]
QUADS = [[0, 2, 4, 6], [1, 3, 5, 7]]


def build_bass(repeat=1):
    nc = bacc.Bacc(num_devices=NCORES)
    xh = nc.declare_dram_parameter("xh", [TH, C], F32, isOutput=False)
    wqkq = nc.declare_dram_parameter("wqkq", [2 * CL // 4, C], F32, isOutput=False)
    wvq = nc.declare_dram_parameter("wvq", [CL // 4, C], F32, isOutput=False)
    wpq = nc.declare_dram_parameter("wpq", [C // 4, CL], F32, isOutput=False)
    bqk = nc.declare_dram_parameter("bqk", [1, 2 * CL], F32, isOutput=False)
    bv = nc.declare_dram_parameter("bv", [1, CL], F32, isOutput=False)
    bp = nc.declare_dram_parameter("bp", [1, C], F32, isOutput=False)
    y_out = nc.declare_dram_parameter("y_out", [TH, C], F32, isOutput=True)

    with tile.TileContext(nc) as tc:
        with (
            tc.tile_pool(name="dram", bufs=1, space="DRAM") as dram,
            tc.tile_pool(name="const", bufs=1) as constp,
            tc.tile_pool(name="wpool", bufs=1) as wpool,
            tc.tile_pool(name="qkv", bufs=1) as qkvp,
            tc.tile_pool(name="nat", bufs=2) as natp,
            tc.tile_pool(name="stg", bufs=2) as stgp,
            tc.tile_pool(name="xch", bufs=2) as xchp,
            tc.tile_pool(name="ptp", bufs=4) as ptp,
            tc.tile_pool(name="otsb", bufs=2) as otsbp,
            tc.tile_pool(name="small", bufs=2) as smallp,
            tc.tile_pool(name="yev", bufs=2) as yevp,
            tc.tile_pool(name="ps_big", bufs=4, space="PSUM") as psb,
            tc.tile_pool(name="ps_ot", bufs=2, space="PSUM") as psot,
            tc.tile_pool(name="ps_bc", bufs=1, space="PSUM") as psbc,
        ):
            # ---- on-device constants ----
            ones128 = constp.tile([128, 128], F32)
            nc.vector.memset(ones128, 1.0)
            ident = constp.tile([128, 128], F32)
            nc.gpsimd.affine_select(
                ident, ones128, pattern=[[1, 128]],
                compare_op=mybir.AluOpType.is_equal, fill=0.0, base=0,
                channel_multiplier=-1)
            # tri[kv, q] = 1 where q >= kv (valid on the diagonal 128 block)
            tri_sb = constp.tile([128, 128], F32)
            nc.gpsimd.affine_select(
                tri_sb, ones128, pattern=[[1, 128]],
                compare_op=mybir.AluOpType.is_ge, fill=0.0, base=0,
                channel_multiplier=-1)
            onesw = constp.tile([1, NQ], F32)
            nc.vector.memset(onesw, 1.0)
            ones_sb = constp.tile([1, NQ], F32R)
            nc.vector.tensor_copy(ones_sb, onesw)
            bqk_sb = constp.tile([1, 2 * CL], F32R)
            nc.sync.dma_start(out=bqk_sb, in_=bqk[:, :].bitcast(F32R))
            bv_sb = constp.tile([1, CL], F32R)
            nc.sync.dma_start(out=bv_sb, in_=bv[:, :].bitcast(F32R))
            bp_sb = constp.tile([1, C], F32R)
            nc.sync.dma_start(out=bp_sb, in_=bp[:, :].bitcast(F32R))

            # ---- persistent SBUF tensors ----
            wqk_sb = [wpool.tile([128, 2 * CL], F32R, tag=f"wqk{cb}",
                                  name=f"wqk{cb}")
                      for cb in range(6)]
            wv_sb = [wpool.tile([128, CL], F32R, tag=f"wv{cb}", name=f"wv{cb}")
                     for cb in range(6)]
            wp_sb = [wpool.tile([128, C], F32R, tag=f"wp{cb}", name=f"wp{cb}")
                     for cb in range(3)]
            QT = [qkvp.tile([128, T], F32R, tag=f"qt{i}", name=f"qt{i}")
                  for i in range(3)]
            KT = [qkvp.tile([128, T], F32R, tag=f"kt{i}", name=f"kt{i}")
                  for i in range(3)]
            V = qkvp.tile([128, NKB, HL, HS + 1], F32R, tag="v")
            nc.vector.tensor_copy(
                V[:, :, :, HS],
                ones128[:, 0:NKB * HL].rearrange("p (a b) -> p a b", b=HL))

            # ---- DRAM scratch ----
            x_bn = dram.tile([TH, C], F32)
            xf = dram.tile([T, C], F32)
            wqk_bn = dram.tile([2 * CL // 4, C], F32)
            wqk_f = dram.tile([2 * CL, C], F32)
            wv_bn = dram.tile([CL // 4, C], F32)
            wv_f = dram.tile([CL, C], F32)
            wp_bn = dram.tile([C // 4, CL], F32)
            wp_f = dram.tile([C, CL], F32)
            xT = dram.tile([C, T], F32)
            y_part = dram.tile([T, C], F32)
            y_rs = dram.tile([TH, C], F32)

            for _rep in range(repeat):
                prep(nc, tc, xh, wqkq, wvq, wpq, x_bn, xf, wqk_bn, wqk_f,
                     wv_bn, wv_f, wp_bn, wp_f, xT, ident, wqk_sb, wv_sb,
                     wp_sb, natp, stgp, psb)
                phase_body(nc, tc, xT, wqk_sb, wv_sb, wp_sb, bqk_sb, bv_sb,
                           bp_sb, tri_sb, ones_sb, QT, KT, V, y_part,
                           xchp, ptp, otsbp, smallp, yevp, psb, psot, psbc)
                nc.gpsimd.collective_compute(
                    "ReduceScatter", mybir.AluOpType.add, replica_groups=PAIRS,
                    ins=[y_part[:, :].opt()], outs=[y_rs[:, :].opt()])
                nc.sync.dma_start(out=y_out[:, :], in_=y_rs[:, :])
    nc.finalize()
    return nc


def prep(nc, tc, xh, wqkq, wvq, wpq, x_bn, xf, wqk_bn, wqk_f, wv_bn, wv_f,
         wp_bn, wp_f, xT, ident, wqk_sb, wv_sb, wp_sb, natp, stgp, psb):
    """Collectives to recover full per-core operands, then PE transposes."""
    engs = [nc.sync, nc.gpsimd, nc.scalar]
    # bounce inputs into collective-legal DRAM, gather
    nc.sync.dma_start(out=x_bn[:, :], in_=xh[:, :])
    nc.gpsimd.collective_compute(
        "AllGather", mybir.AluOpType.bypass, replica_groups=PAIRS,
        ins=[x_bn[:, :].opt()], outs=[xf[:, :].opt()])
    nc.scalar.dma_start(out=wqk_bn[:, :], in_=wqkq[:, :])
    nc.gpsimd.collective_compute(
        "AllGather", mybir.AluOpType.bypass, replica_groups=QUADS,
        ins=[wqk_bn[:, :].opt()], outs=[wqk_f[:, :].opt()])
    nc.scalar.dma_start(out=wv_bn[:, :], in_=wvq[:, :])
    nc.gpsimd.collective_compute(
        "AllGather", mybir.AluOpType.bypass, replica_groups=QUADS,
        ins=[wv_bn[:, :].opt()], outs=[wv_f[:, :].opt()])
    nc.scalar.dma_start(out=wp_bn[:, :], in_=wpq[:, :])
    nc.gpsimd.collective_compute(
        "AllGather", mybir.AluOpType.bypass, replica_groups=QUADS,
        ins=[wp_bn[:, :].opt()], outs=[wp_f[:, :].opt()])

    # x transpose: [T, C] -> xT [C, T], in [256 t x 128 c] PE tiles
    for g2 in range(8):
        xna = natp.tile([128, 2, C], F32, tag="xna")
        engs[g2 % 3].dma_start(
            out=xna,
            in_=xf[g2 * 256:(g2 + 1) * 256, :].rearrange(
                "(a p) c -> p a c", p=128))
        for cb in range(6):
            ps = psb.tile([128, NQ], F32, tag="big")
            for j in range(2):
                nc.tensor.transpose(
                    ps[:, j * 128:(j + 1) * 128],
                    xna[:, j, cb * 128:(cb + 1) * 128], ident)
            st = stgp.tile([128, 256], F32, tag="xst")
            nc.vector.tensor_copy(st, ps[:, 0:256])
            engs[cb % 3].dma_start(
                out=xT[cb * 128:(cb + 1) * 128, g2 * 256:(g2 + 1) * 256],
                in_=st)

    # weight transposes straight into persistent SBUF (as f32r bits)
    for rb in range(6):
        wna = natp.tile([128, C], F32, tag="wna")
        engs[rb % 3].dma_start(out=wna, in_=wqk_f[rb * 128:(rb + 1) * 128, :])
        for cb in range(6):
            ps = psb.tile([128, NQ], F32, tag="big")
            nc.tensor.transpose(
                ps[:, 0:128], wna[:, cb * 128:(cb + 1) * 128], ident)
            nc.vector.tensor_copy(
                wqk_sb[cb][:, rb * 128:(rb + 1) * 128], ps[:, 0:128])
    for rb in range(3):
        wna = natp.tile([128, C], F32, tag="wna")
        engs[rb % 3].dma_start(out=wna, in_=wv_f[rb * 128:(rb + 1) * 128, :])
        for cb in range(6):
            ps = psb.tile([128, NQ], F32, tag="big")
            nc.tensor.transpose(
                ps[:, 0:128], wna[:, cb * 128:(cb + 1) * 128], ident)
            nc.vector.tensor_copy(
                wv_sb[cb][:, rb * 128:(rb + 1) * 128], ps[:, 0:128])
    for ob in range(6):
        wna = natp.tile([128, CL], F32, tag="wpa")
        engs[ob % 3].dma_start(out=wna, in_=wp_f[ob * 128:(ob + 1) * 128, :])
        for cb in range(3):
            ps = psb.tile([128, NQ], F32, tag="big")
            nc.tensor.transpose(
                ps[:, 0:128], wna[:, cb * 128:(cb + 1) * 128], ident)
            nc.vector.tensor_copy(
                wp_sb[cb][:, ob * 128:(ob + 1) * 128], ps[:, 0:128])


def phase_body(nc, tc, xT, wqk_sb, wv_sb, wp_sb, bqk_sb, bv_sb, bp_sb, tri_sb,
               ones_sb, QT, KT, V, y_part,
               xchp, ptp, otsbp, smallp, yevp, psb, psot, psbc):
    engs = [nc.sync, nc.gpsimd, nc.scalar]
    xTr = xT[:, :].bitcast(F32R).rearrange("(cb p) t -> p cb t", p=128)

    # ---- Phase A: QKV projection per t-chunk ----
    for tcn in range(NCH):
        xc = xchp.tile([128, 6, NQ], F32R, tag="xc")
        for cb in range(6):
            engs[(cb + tcn) % 3].dma_start(
                out=xc[:, cb, :],
                in_=xTr[:, cb, tcn * NQ:(tcn + 1) * NQ])
        # Q^T / K^T: [r, t] layout, 6 row-blocks (3 Q + 3 K)
        for rb in range(6):
            ps = psb.tile([128, NQ], F32, tag="big")
            for cb in range(6):
                nc.tensor.matmul(
                    ps, lhsT=wqk_sb[cb][:, rb * 128:(rb + 1) * 128],
                    rhs=xc[:, cb, :], start=(cb == 0), stop=False)
            nc.tensor.matmul(
                ps, lhsT=bqk_sb[:, rb * 128:(rb + 1) * 128],
                rhs=ones_sb, start=False, stop=True)
            dst = QT[rb] if rb < 3 else KT[rb - 3]
            nc.vector.tensor_copy(dst[:, tcn * NQ:(tcn + 1) * NQ], ps)
        # V: [t, r] layout, 4 t-subblocks
        for tb in range(4):
            ti = tcn * 4 + tb
            psv = psb.tile([128, CL], F32, tag="big")
            for cb in range(6):
                nc.tensor.matmul(
                    psv, lhsT=xc[:, cb, tb * 128:(tb + 1) * 128],
                    rhs=wv_sb[cb], start=(cb == 0), stop=False)
            nc.tensor.matmul(
                psv, lhsT=ones_sb[:, 0:128], rhs=bv_sb,
                start=False, stop=True)
            nc.vector.tensor_copy(
                V[:, ti, :, 0:HS],
                psv.rearrange("p (h d) -> p h d", d=HS))

    # ---- Phase B: attention + c_proj per q-block ----
    for J in range(NCH):
        qs = slice(J * NQ, (J + 1) * NQ)
        ots = [otsbp.tile([128, NQ], F32R, tag=f"ots{cb}", name=f"ots{cb}")
               for cb in range(3)]
        for h in range(HL):
            kb, po = h // 2, (h % 2) * HS
            qt = QT[kb][po:po + HS, qs]
            ot = psot.tile([HS + 1, NQ], F32, tag="ot")
            # software pipeline: issue S(t)/exp(t), then O(t-1), so the
            # PE works on S(t) while ACT computes exp(t-1).
            pending = None   # (o_args, kwargs) for the deferred O matmul
            for t in range(J * 4 + 4):
                diag = t - J * 4             # >= 0 on diagonal tiles
                sps = psb.tile([128, NQ], F32, tag="big")
                pt = ptp.tile([128, NQ], F32R, tag="pt")
                if diag < 0:                 # full kv tile
                    nc.tensor.matmul(
                        sps, lhsT=KT[kb][po:po + HS, t * 128:(t + 1) * 128],
                        rhs=qt, start=True, stop=True)
                    nc.scalar.activation(pt, sps, EXP, scale=0.125)
                    omm = dict(out=ot, lhsT=V[:, t, h, :], rhs=pt,
                               start=(t == 0), stop=False)
                else:
                    W = NQ - 128 * diag
                    nc.tensor.matmul(
                        sps[:, 0:W],
                        lhsT=KT[kb][po:po + HS, t * 128:(t + 1) * 128],
                        rhs=QT[kb][po:po + HS,
                                   J * NQ + 128 * diag:(J + 1) * NQ],
                        start=True, stop=True)
                    nc.scalar.activation(pt[:, 0:W], sps[:, 0:W], EXP,
                                         scale=0.125)
                    nc.vector.tensor_mul(pt[:, 0:128], pt[:, 0:128],
                                         tri_sb)
                    omm = dict(out=ot[:, 128 * diag:NQ],
                               lhsT=V[:, t, h, :], rhs=pt[:, 0:W],
                               start=(J == 0 and diag == 0), stop=False)
                if pending is not None:
                    o = pending
                    nc.tensor.matmul(o.pop("out"), **o)
                pending = omm
            pending["stop"] = True
            o = pending
            nc.tensor.matmul(o.pop("out"), **o)
            # normalize: recip of denominator row, broadcast via rank-1 mm
            rec = smallp.tile([1, NQ], F32R, tag="rec")
            with nc.allow_low_precision(reason="fp32r matmul operand"):
                nc.vector.reciprocal(rec, ot[HS:HS + 1, :])
            bc = psbc.tile([HS, NQ], F32, tag="bc")
            nc.tensor.matmul(bc, lhsT=ones_sb[:, 0:HS], rhs=rec,
                             start=True, stop=True)
            bcs = smallp.tile([HS, NQ], F32, tag="bcs")
            nc.vector.tensor_copy(bcs, bc)
            nc.vector.tensor_mul(ots[kb][po:po + HS, :], ot[0:HS, :], bcs)
        # c_proj for this q-block (+ bias, zero on odd cores)
        for i in range(4):
            ti = J * 4 + i
            yt = yevp.tile([128, C], F32, tag="yt")
            for half in range(2):
                yps = psb.tile([128, CL], F32, tag="yps", bufs=1)
                for cb in range(3):
                    nc.tensor.matmul(
                        yps, lhsT=ots[cb][:, i * 128:(i + 1) * 128],
                        rhs=wp_sb[cb][:, half * CL:(half + 1) * CL],
                        start=(cb == 0), stop=False)
                nc.tensor.matmul(
                    yps, lhsT=ones_sb[:, 0:128],
                    rhs=bp_sb[:, half * CL:(half + 1) * CL],
                    start=False, stop=True)
                nc.vector.tensor_copy(yt[:, half * CL:(half + 1) * CL], yps)
            nc.sync.dma_start(
                out=y_part[ti * 128:(ti + 1) * 128, :], in_=yt)


_ZERO_BIAS = np.zeros((1, C), dtype=np.float32)


def make_in_maps(x, w_attn, b_attn, w_proj, b_proj=None):
    """Build per-core inputs. Every large entry is a VIEW of the originals."""
    x = np.asarray(x, dtype=np.float32)
    w_attn = np.asarray(w_attn, dtype=np.float32)
    b_attn = np.asarray(b_attn, dtype=np.float32)
    w_proj = np.asarray(w_proj, dtype=np.float32)
    xr = x.reshape(B * T, C)
    bp_full = (np.asarray(b_proj, dtype=np.float32).reshape(1, C)
               if b_proj is not None else _ZERO_BIAS)
    in_maps = []
    for core in range(NCORES):
        b, g = divmod(core, 2)
        sl0 = g * CL
        # quarter b of the stacked [wq_slice; wk_slice] (2*CL=768 rows)
        if b < 2:
            wqkq = w_attn[sl0 + b * 192:sl0 + (b + 1) * 192]
        else:
            wqkq = w_attn[C + sl0 + (b - 2) * 192:C + sl0 + (b - 1) * 192]
        in_maps.append({
            "xh": xr[b * T + g * TH:b * T + (g + 1) * TH],
            "wqkq": wqkq,
            "wvq": w_attn[2 * C + sl0 + b * 96:2 * C + sl0 + (b + 1) * 96],
            "wpq": w_proj[b * 192:(b + 1) * 192, sl0:sl0 + CL],
            "bqk": np.concatenate(
                [b_attn[sl0:sl0 + CL], b_attn[C + sl0:C + sl0 + CL]])[None, :],
            "bv": b_attn[2 * C + sl0:2 * C + sl0 + CL][None, :],
            "bp": bp_full if g == 0 else _ZERO_BIAS,
        })
    return in_maps


def assemble(results):
    ys = [r["y_out"] for r in results]
    # run_bass_via_pjrt returns per-core views of one (NCORES*TH, C) array;
    # reuse it zero-copy when possible.
    base = ys[0].base
    while base is not None and base.base is not None:
        base = base.base
    if (base is not None and base.size == NCORES * TH * C
            and base.dtype == np.float32
            and all(np.shares_memory(yc, base) for yc in ys)
            and all(yc.__array_interface__["data"][0]
                    == base.__array_interface__["data"][0] + c * TH * C * 4
                    for c, yc in enumerate(ys))):
        return base.reshape(B, T, C)
    return np.concatenate(ys, axis=0).reshape(B, T, C)


_CACHE = {}


def _get_nc():
    if "nc" not in _CACHE:
        _CACHE["nc"] = build_bass()
    return _CACHE["nc"]


def kernel(x, w_attn, b_attn, w_proj, b_proj):
    in_maps = make_in_maps(x, w_attn, b_attn, w_proj, b_proj)
    res = run_bass_kernel_spmd(_get_nc(), in_maps, list(range(NCORES)))
    return assemble(res.results)
